# revision 1
# baseline (speedup 1.0000x reference)
"""DRew-GIN message passing on 8 Trainium2 NeuronCores.

Sharding: nodes are permuted into 8 cores x 49 tiles of 128 (degree-balanced
bins). Edges are partitioned by edge type k and by the owner of their dst
node, so segment_sum is local to each core. Delayed source features
xs[t-delay] are all-gathered between layers; the d x d MLP weights are
replicated.

Device kernel per core/layer t:
  out = relu(xT_local @ Ws + bs)                    (dense, PE + ACT)
  for k in 1..t+1:
    msg   = gather rows of xs[t-k+1] by src         (batched indirect DMA)
    aggT  = sum_e msg_e onehot(dst_e)               (selection-matmul on PE,
                                                     accumulated in PSUM)
    out  += relu(Wk.T @ aggT + bk)                  (PE + ACT + DVE)
  x_next = x + relu(out)                            (residual, DVE)
  AllGather(x_next rows) -> next layer's gather table
"""

import numpy as np

N = 50000
E = 800000
D = 128
L = 3
NCORES = 8
P = 128
TILES_PER_CORE = 49          # ceil(6250 / 128)
NPC = TILES_PER_CORE * P     # padded nodes per core = 6272
NPAD = NCORES * NPC          # padded total rows = 50176
NBINS = NCORES * TILES_PER_CORE
BT = 32                      # gather batch, in edge tiles
PAD_DST = 999.0

_CACHE = {}


def _balance_nodes(deg_total):
    """Assign nodes to NBINS bins of <=128 slots, snake order by degree."""
    order = np.argsort(-deg_total, kind="stable")
    bin_of = np.empty(N, dtype=np.int64)
    slot_of = np.empty(N, dtype=np.int64)
    fill = np.zeros(NBINS, dtype=np.int64)
    pos = 0
    rnd = 0
    while pos < N:
        take = min(NBINS, N - pos)
        bins = np.arange(NBINS) if rnd % 2 == 0 else np.arange(NBINS)[::-1]
        bins = bins[:take]
        nodes = order[pos : pos + take]
        bin_of[nodes] = bins
        slot_of[nodes] = fill[bins]
        fill[bins] += 1
        pos += take
        rnd += 1
    assert fill.max() <= P
    return bin_of, slot_of


def _prep(x, edge_index, edge_attr, Ws, bs, Wk, bk, eps):
    x = np.asarray(x, dtype=np.float32)
    ei = np.asarray(edge_index).astype(np.int64)
    ea = np.asarray(edge_attr).astype(np.int64)
    Ws = np.asarray(Ws, dtype=np.float32)
    bs = np.asarray(bs, dtype=np.float32)
    Wk = np.asarray(Wk, dtype=np.float32)
    bk = np.asarray(bk, dtype=np.float32)
    eps = np.asarray(eps, dtype=np.float32)
    src, dst = ei[0], ei[1]

    # per-k in-degree and balanced node assignment
    degs = [np.bincount(dst[ea == k], minlength=N) for k in range(1, L + 1)]
    bin_of, slot_of = _balance_nodes(sum(degs))
    core_of_bin = np.arange(NBINS) // TILES_PER_CORE
    tile_of_bin = np.arange(NBINS) % TILES_PER_CORE
    rowid = core_of_bin[bin_of] * NPC + tile_of_bin[bin_of] * P + slot_of

    # padded, permuted feature table
    x0p = np.zeros((NPAD, D), dtype=np.float32)
    x0p[rowid] = x

    # edge layout per k: groups = (core, tile-of-dst); fixed NT tiles/group
    src_arrs, dst_arrs, NTs = [], [], []
    for k in range(1, L + 1):
        m = ea == k
        s_k, d_k = src[m], dst[m]
        g = bin_of[d_k]  # group 0..NBINS-1
        cnt = np.bincount(g, minlength=NBINS)
        NT = int(np.ceil(cnt.max() / P))
        Tk = TILES_PER_CORE * NT
        o = np.argsort(g, kind="stable")
        s_k, d_k, g = s_k[o], d_k[o], g[o]
        starts = np.zeros(NBINS, dtype=np.int64)
        starts[1:] = np.cumsum(cnt)[:-1]
        within = np.arange(len(g)) - starts[g]  # slot within group
        # slot in the padded per-core stream
        gslot = (g % TILES_PER_CORE) * NT * P + within
        core = g // TILES_PER_CORE
        A_src = np.zeros((NCORES, P, Tk), dtype=np.int32)
        A_dst = np.full((NCORES, P, Tk), PAD_DST, dtype=np.float32)
        tile_j = gslot // P
        part_p = gslot % P
        A_src[core, part_p, tile_j] = rowid[s_k].astype(np.int32)
        A_dst[core, part_p, tile_j] = slot_of[d_k].astype(np.float32)
        src_arrs.append(A_src)
        dst_arrs.append(A_dst)
        NTs.append(NT)

    # fold (1+eps) into Ws/bs when nonnegative
    scale = 1.0 + eps
    Ws_eff = Ws.copy()
    bs_eff = bs.copy()
    post_scale = [None] * L
    for t in range(L):
        if scale[t] >= 0.0:
            Ws_eff[t] *= scale[t]
            bs_eff[t] *= scale[t]
        else:
            post_scale[t] = float(scale[t])

    # packed weight mats: 0..2 = Ws_eff[t]; 3.. = Wk[t, k-1] for used (t,k)
    wk_idx = {}
    mats = [Ws_eff[0], Ws_eff[1], Ws_eff[2]]
    for t in range(L):
        for k in range(1, t + 2):
            wk_idx[(t, k)] = len(mats)
            mats.append(Wk[t, k - 1])
    wmats = np.stack(mats).astype(np.float32)  # [9, 128, 128]

    bcols = np.zeros((P, 12), dtype=np.float32)
    for t in range(L):
        bcols[:, t] = bs_eff[t]
        for k in range(1, t + 2):
            bcols[:, 3 + 3 * t + (k - 1)] = bk[t, k - 1]

    return dict(
        x0p=x0p, rowid=rowid, src_arrs=src_arrs, dst_arrs=dst_arrs,
        NTs=NTs, wmats=wmats, bcols=bcols, wk_idx=wk_idx,
        post_scale=post_scale,
    )


def _patch_tile_drain(tile_mod):
    """walrus in this toolchain rejects >2 sem waits on one instruction; split
    the TileContext exit drain's waits across extra drain instructions."""
    import bass_rust

    if getattr(tile_mod.TileContext, "_drain_patched", False):
        return
    orig = tile_mod.TileContext._drain_and_barrier

    def patched(self, tick_clock, wait_clock):
        nc = self.nc
        drain_inst = nc.sync.drain()
        wait_clock.add_sem_waits(
            drain_inst.ins,
            tile_mod.ScopedClock({None: tick_clock.global_clock}),
        )
        si = drain_inst.ins.sync_info
        if si is not None and si.on_wait is not None and len(si.on_wait) > 2:
            waits = list(si.on_wait)
            drain_inst.ins.sync_info = bass_rust.SyncInfo(
                on_wait=waits[:2], on_update=list(si.on_update or [])
            )
            for i in range(2, len(waits), 2):
                extra = nc.sync.drain()
                extra.ins.sync_info = bass_rust.SyncInfo(
                    on_wait=waits[i : i + 2], on_update=[]
                )
        nc.all_engine_barrier()
        assert self.sems is not None
        popped = nc._tile_sem_poison_stack.pop()
        assert popped is self._sem_poison
        nc.clear_and_free_semaphores(list(self.sems.allocated().values()))
        nc.all_engine_barrier()

    tile_mod.TileContext._drain_and_barrier = patched
    tile_mod.TileContext._drain_patched = True
    tile_mod.TileContext._drain_orig = orig


def _split_excess_waits(nc, mybir, max_waits=1):
    """This walrus build rejects >1 sem wait per instruction; hoist excess
    waits onto event-semaphore instructions inserted just before, on the
    same engine (keeps original wait position -> no deadlock risk)."""
    import bass_rust

    for fn in nc.m.functions:
        for bb in fn.blocks:
            new_insts = []
            for ins in bb.instructions:
                si = ins.sync_info
                if si is not None and si.on_wait and len(si.on_wait) > max_waits:
                    waits = list(si.on_wait)
                    extra, keep = waits[:-max_waits], waits[-max_waits:]
                    for i, w in enumerate(extra):
                        ev = mybir.InstEventSemaphore(
                            name=f"{ins.name}-xw{i}",
                            sync_info=bass_rust.SyncInfo(
                                on_wait=[w], on_update=[]
                            ),
                            engine=ins.engine,
                        )
                        new_insts.append(ev)
                    ins.sync_info = bass_rust.SyncInfo(
                        on_wait=keep, on_update=list(si.on_update or [])
                    )
                new_insts.append(ins)
            bb.instructions[:] = new_insts


def _build(NTs, wk_idx, post_scale, use_f32r=True, n_layers=L):
    import concourse.bass as bass
    import concourse.mybir as mybir
    import concourse.tile as tile
    from concourse.masks import make_identity

    _patch_tile_drain(tile)

    f32 = mybir.dt.float32
    f32r = mybir.dt.float32r
    nc = bass.Bass("TRN2", target_bir_lowering=False, debug=False,
                   num_devices=NCORES)

    x0_tbl = nc.dram_tensor("x0_tbl", [NPAD, D], f32, kind="ExternalInput")
    x0_loc = nc.dram_tensor("x0_loc", [NPC, D], f32, kind="ExternalInput")
    wmats_d = nc.dram_tensor("wmats", [9, P, D], f32, kind="ExternalInput")
    bcols_d = nc.dram_tensor("bcols", [P, 12], f32, kind="ExternalInput")
    srcs_d, dsts_d = [], []
    for k in range(1, L + 1):
        Tk = TILES_PER_CORE * NTs[k - 1]
        srcs_d.append(nc.dram_tensor(f"srcs{k}", [P, Tk], mybir.dt.int32,
                                     kind="ExternalInput"))
        dsts_d.append(nc.dram_tensor(f"dsts{k}", [P, Tk], f32,
                                     kind="ExternalInput"))
    out_loc = nc.dram_tensor("out_loc", [NPC, D], f32, kind="ExternalOutput")

    bounce = [None,
              nc.dram_tensor("b1", [NPC, D], f32),
              nc.dram_tensor("b2", [NPC, D], f32)]
    tables = [x0_tbl,
              nc.dram_tensor("t1", [NPAD, D], f32, addr_space="Shared"),
              nc.dram_tensor("t2", [NPAD, D], f32, addr_space="Shared")]

    NW = 13  # 12 windows of 512 + 1 of 128
    WIDTHS = [512] * 12 + [128]

    def mm_cast(ap):
        return ap.bitcast(f32r) if use_f32r else ap

    with tile.TileContext(nc) as tc:
        with (
            tc.tile_pool(name="pers", bufs=1) as pers,
            tc.tile_pool(name="sb", bufs=2) as sb,
            tc.tile_pool(name="sel", bufs=6) as selp,
            tc.tile_pool(name="ps", bufs=2, space="PSUM") as ps,
        ):
            ident = pers.tile([P, P], f32)
            make_identity(nc, ident[:])
            iota_i = pers.tile([P, P], mybir.dt.int32)
            nc.gpsimd.iota(iota_i[:], pattern=[[1, P]], channel_multiplier=0)
            iota = pers.tile([P, P], f32)
            nc.vector.tensor_copy(iota[:], iota_i[:])

            wm = pers.tile([P, 9 * P], f32)
            for m in range(9):
                nc.sync.dma_start(wm[:, m * P : (m + 1) * P], wmats_d[m])
            bc = pers.tile([P, 12], f32)
            nc.sync.dma_start(bc[:], bcols_d[:])

            xTa = pers.tile([P, NPC], f32)
            xTb = pers.tile([P, NPC], f32)
            out_sb = pers.tile([P, NPC], f32)

            # init xTa = x0_loc transposed
            for g in range(TILES_PER_CORE):
                rows = sb.tile([P, P], f32, tag="rows")
                nc.sync.dma_start(rows[:], x0_loc[g * P : (g + 1) * P, :])
                tp = ps.tile([P, P], f32, space="PSUM", tag="tp")
                nc.tensor.transpose(out=tp[:], in_=rows[:], identity=ident[:])
                nc.vector.tensor_copy(xTa[:, g * P : (g + 1) * P], tp[:])

            xs = [xTa, xTb, xTa, xTb]

            def sweep(t, k):
                tbl = tables[t - (k - 1)]
                NT = NTs[k - 1]
                Tk = TILES_PER_CORE * NT
                ssb = sb.tile([P, Tk], mybir.dt.int32, tag="srcs")
                dsb = sb.tile([P, Tk], f32, tag="dsts")
                nc.sync.dma_start(ssb[:], srcs_d[k - 1][:])
                nc.sync.dma_start(dsb[:], dsts_d[k - 1][:])
                kcol = bc[:, 3 + 3 * t + (k - 1) : 4 + 3 * t + (k - 1)]
                wk_sl = wk_idx[(t, k)]
                wk_ap = wm[:, wk_sl * P : (wk_sl + 1) * P]
                cur_ps = None
                for b0 in range(0, Tk, BT):
                    bw = min(BT, Tk - b0)
                    # HW indirect DMA honors one index per partition, so
                    # gather per edge tile ([128,1] offsets, [128,128] rows)
                    msgs = []
                    for j in range(bw):
                        q = b0 + j
                        mt = sb.tile([P, P], f32, tag="msg")
                        nc.gpsimd.indirect_dma_start(
                            out=mt[:],
                            out_offset=None,
                            in_=tbl[:],
                            in_offset=bass.IndirectOffsetOnAxis(
                                ap=ssb[:, q : q + 1], axis=0),
                        )
                        msgs.append(mt)
                    for j in range(bw):
                        q = b0 + j
                        g = q // NT
                        W, sl = g // 4, g % 4
                        first = q % NT == 0
                        last = q % NT == NT - 1
                        if first and sl == 0:
                            cur_ps = ps.tile([P, 512], f32, space="PSUM",
                                             tag="aggT")
                        sel = selp.tile([P, P], f32, tag="sel")
                        nc.vector.tensor_scalar(
                            out=sel[:], in0=iota[:],
                            scalar1=dsb[:, q : q + 1], scalar2=None,
                            op0=mybir.AluOpType.is_equal)
                        nc.tensor.matmul(
                            out=cur_ps[:, sl * P : (sl + 1) * P],
                            lhsT=msgs[j][:],
                            rhs=sel[:], start=first, stop=last)
                        if last and (sl == 3 or g == TILES_PER_CORE - 1):
                            w = WIDTHS[W]
                            aggT = sb.tile([P, 512], f32, tag="aggTsb")
                            nc.vector.tensor_copy(aggT[:, :w], cur_ps[:, :w])
                            oT = ps.tile([P, 512], f32, space="PSUM",
                                         tag="outT")
                            nc.tensor.matmul(
                                out=oT[:, :w], lhsT=mm_cast(wk_ap),
                                rhs=mm_cast(aggT[:, :w]),
                                start=True, stop=True)
                            tmp = sb.tile([P, 512], f32, tag="ktmp")
                            nc.scalar.activation(
                                out=tmp[:, :w], in_=oT[:, :w],
                                func=mybir.ActivationFunctionType.Relu,
                                bias=kcol)
                            osl = out_sb[:, W * 512 : W * 512 + w]
                            nc.vector.tensor_add(out=osl, in0=osl,
                                                 in1=tmp[:, :w])

            for t in range(n_layers):
                xT_cur, xT_next = xs[t], xs[t + 1]
                if t > 0:
                    nc.gpsimd.collective_compute(
                        "AllGather", mybir.AluOpType.bypass,
                        ins=[bounce[t][:]], outs=[tables[t][:]],
                        replica_groups=[list(range(NCORES))])
                # mlp_s into out_sb (first term writes directly)
                for W in range(NW):
                    w = WIDTHS[W]
                    xsl = xT_cur[:, W * 512 : W * 512 + w]
                    oT = ps.tile([P, 512], f32, space="PSUM", tag="outT")
                    nc.tensor.matmul(out=oT[:, :w],
                                     lhsT=mm_cast(wm[:, t * P : (t + 1) * P]),
                                     rhs=mm_cast(xsl), start=True, stop=True)
                    osl = out_sb[:, W * 512 : W * 512 + w]
                    if post_scale[t] is None:
                        nc.scalar.activation(
                            out=osl, in_=oT[:, :w],
                            func=mybir.ActivationFunctionType.Relu,
                            bias=bc[:, t : t + 1])
                    else:
                        tmp = sb.tile([P, 512], f32, tag="ktmp")
                        nc.scalar.activation(
                            out=tmp[:, :w], in_=oT[:, :w],
                            func=mybir.ActivationFunctionType.Relu,
                            bias=bc[:, t : t + 1])
                        nc.vector.tensor_scalar(
                            out=osl, in0=tmp[:, :w],
                            scalar1=post_scale[t], scalar2=None,
                            op0=mybir.AluOpType.mult)
                # old-table sweeps first, AG-dependent k=1 last
                for k in range(t + 1, 0, -1):
                    sweep(t, k)
                # finalize: h = relu(out); x_next = x_cur + h; emit rows
                for W in range(NW):
                    w = WIDTHS[W]
                    osl = out_sb[:, W * 512 : W * 512 + w]
                    h = sb.tile([P, 512], f32, tag="h")
                    nc.scalar.activation(
                        out=h[:, :w], in_=osl,
                        func=mybir.ActivationFunctionType.Relu)
                    nc.vector.tensor_add(
                        out=xT_next[:, W * 512 : W * 512 + w],
                        in0=xT_cur[:, W * 512 : W * 512 + w], in1=h[:, :w])
                dst_rows = out_loc if t == n_layers - 1 else bounce[t + 1]
                for g in range(TILES_PER_CORE):
                    tp = ps.tile([P, P], f32, space="PSUM", tag="tp")
                    nc.tensor.transpose(
                        out=tp[:], in_=xT_next[:, g * P : (g + 1) * P],
                        identity=ident[:])
                    rows = sb.tile([P, P], f32, tag="rows")
                    nc.vector.tensor_copy(rows[:], tp[:])
                    nc.sync.dma_start(dst_rows[g * P : (g + 1) * P, :],
                                      rows[:])
    _split_excess_waits(nc, mybir)
    return nc


def kernel(x, edge_index, edge_attr, Ws, bs, Wk, bk, eps):
    from concourse.bass_utils import run_bass_kernel_spmd

    prep = _prep(x, edge_index, edge_attr, Ws, bs, Wk, bk, eps)
    key = (tuple(prep["NTs"]), tuple(prep["post_scale"]))
    if key not in _CACHE:
        _CACHE[key] = _build(prep["NTs"], prep["wk_idx"], prep["post_scale"],
                             use_f32r=False)
    nc = _CACHE[key]

    in_maps = []
    for c in range(NCORES):
        m = dict(
            x0_tbl=prep["x0p"],
            x0_loc=np.ascontiguousarray(prep["x0p"][c * NPC : (c + 1) * NPC]),
            wmats=prep["wmats"],
            bcols=prep["bcols"],
        )
        for k in range(1, L + 1):
            m[f"srcs{k}"] = np.ascontiguousarray(prep["src_arrs"][k - 1][c])
            m[f"dsts{k}"] = np.ascontiguousarray(prep["dst_arrs"][k - 1][c])
        in_maps.append(m)

    res = run_bass_kernel_spmd(nc, in_maps, list(range(NCORES)))
    out_pad = np.concatenate(
        [res.results[c]["out_loc"] for c in range(NCORES)], axis=0)
    out = out_pad[prep["rowid"]]
    return out.astype(np.float32)

